# revision 3
# baseline (speedup 1.0000x reference)
# Trainium2 Bass kernel for nn_DiversityLoss (segment_reduce).
#
# reference:
#   sums   = segment_sum(embeddings, labels, C)        # [C, D]
#   counts = segment_sum(ones, labels, C)              # [C]
#   return -mean(var(sums / counts, axis=0, ddof=1))
#
# Strategy v2 (sorted layout, fp8, bucket-scheduled):
#   The v1 kernel was PE-bound: an unsorted 128-row tile can hit any of the
#   1000 classes, so exact per-class sums need a 1000-wide one-hot matmul
#   (~417 ns/tile).  Host-side LAYOUT work removes that: permute rows so
#   that each 128-row tile touches at most 4 consecutive class slots, then
#   the per-tile matmul is LDWEIGHTS(emb 128x128 fp8, fast-weight-load) +
#   a 4-column matmul -- tens of ns instead of 417.
#
#   - Classes are bin-packed into 504 buckets (8 singles for the largest
#     classes + 496 two-pointer pairs), every bucket padded to the max
#     bucket size R2 (~1% pad).  Core k owns buckets [63k, 63k+63): the
#     tile -> psum-column schedule c0(t) = 2*floor(128t/R2) is then
#     label-independent and identical on all 8 cores (SPMD requirement).
#   - Per tile t: matmul(psum[:, c0:c0+4], lhsT=emb_tile[128,128] fp8,
#     rhs=indicator[128,4] fp8).  The indicator (which of the 4 slots each
#     row belongs to) is built on host as tiny fp8 data (~3% of emb bytes).
#   - PSUM [128 dims, 128 slots] fp32 accumulates everything; one zeroing
#     matmul opens the accumulation group, one closes it.
#   - Host: map (core, slot) -> class, divide by bincount counts, variance
#     in float64.  Embeddings are cast fp32->fp16->fp8e4m3 via a 64K-entry
#     LUT (adds ~0.1% relative error to the final variance, tolerance 2e-2).
#
# Expected: DMA ~16.2 MB/core fp8 at ~360-420 GB/s ~= 40-45 us, PE ~987
# tiles at ~30-60 ns ~= 30-60 us, overlapped.

import numpy as np
import ml_dtypes

N = 1_000_000
D = 128
C = 1000
CORES = 8
NB_PER_CORE = 63
NB = NB_PER_CORE * CORES  # 504 buckets, <=2 classes each
W = 4  # indicator window width (psum columns per matmul)

F8 = ml_dtypes.float8_e4m3

# test.py can flip this before calling kernel() to capture a profile; the
# BassKernelResults of the last run is stored in LAST_RESULT either way.
TRACE = False
TRACE_KWARGS = {}
LAST_RESULT = None

_cached_nc = {}
_fp8_lut = None


def _lut():
    global _fp8_lut
    if _fp8_lut is None:
        with np.errstate(invalid="ignore", over="ignore"):
            _fp8_lut = (
                np.arange(65536, dtype=np.uint16)
                .view(np.float16)
                .astype(F8)
                .view(np.uint8)
            )
    return _fp8_lut


def _pack_classes(counts):
    """Pack C classes into NB buckets of <=2 classes; returns (buckets, R2).

    8 largest classes go in single buckets; the remaining 992 are paired
    largest-with-smallest, which keeps pair sums tight around 2*mean.
    R2 = max bucket row count = the padded per-bucket size.
    """
    n_singles = 2 * NB - C  # 8
    order = np.argsort(counts, kind="stable")[::-1]
    buckets = [[int(c)] for c in order[:n_singles]]
    rest = order[n_singles:]
    half = len(rest) // 2
    for i in range(half):
        buckets.append([int(rest[i]), int(rest[len(rest) - 1 - i])])
    sums = [int(sum(counts[c] for c in b)) for b in buckets]
    R2 = max(max(sums), 2 * 128)
    return buckets, R2


def _schedule(T, R2):
    # psum column window base per tile; identical on every core.
    return [min(2 * ((128 * t) // R2), 128 - W) for t in range(T)]


def _build_module(T, R2):
    import concourse.mybir as mybir
    import concourse.tile as tile
    from concourse import bacc

    f8 = mybir.dt.float8e4
    f32 = mybir.dt.float32
    c0s = _schedule(T, R2)

    nc = bacc.Bacc(
        "TRN2",
        target_bir_lowering=False,
        debug=False,
        enable_asserts=False,
        num_devices=CORES,
    )
    emb_d = nc.dram_tensor("emb", [128, T * D], f8, kind="ExternalInput")
    ind_d = nc.dram_tensor("ind", [128, T * W], f8, kind="ExternalInput")
    out_d = nc.dram_tensor("out", [128, 128], f32, kind="ExternalOutput")

    with tile.TileContext(nc) as tc:
        with (
            tc.tile_pool(name="consts", bufs=1) as consts,
            tc.tile_pool(name="psum", bufs=1, space="PSUM") as psum,
        ):
            et = consts.tile([128, T * D], f8)
            ind_t = consts.tile([128, T * W], f8)
            zero8 = consts.tile([128, 128], f8)
            out_t = consts.tile([128, 128], f32)
            ps = psum.tile([128, 128], f32)

            nc.vector.memset(zero8[:], 0.0)

            # emb chunk 0 is the very first transfer on the sync ring (it
            # gates the first matmuls); the indicators ride the scalar
            # HWDGE ring so their descriptor generation overlaps.
            IND1 = min(256, T)
            nc.scalar.dma_start(out=ind_t[:, 0 : IND1 * W], in_=ind_d[:, 0 : IND1 * W])

            # Small chunks first (compute starts early), 128-tile chunks in
            # the middle, and a small final chunk so the compute tail after
            # the last DMA semaphore is short.
            splits = [0, 8, 32, 128]
            while splits[-1] < T - 154:
                splits.append(splits[-1] + 128)
            splits.extend([T - 26, T])
            splits = sorted(set(s for s in splits if s <= T))

            # Open the accumulation group: zero the whole [128,128] psum
            # region so every later matmul accumulates (per-element
            # has_written) regardless of which columns it touches.
            nc.tensor.matmul(
                ps[:], lhsT=zero8[:], rhs=zero8[:], start=True, stop=False
            )
            for ch in range(len(splits) - 1):
                t0, t1 = splits[ch], splits[ch + 1]
                nc.sync.dma_start(
                    out=et[:, t0 * D : t1 * D], in_=emb_d[:, t0 * D : t1 * D]
                )
                if ch == 1 and IND1 < T:
                    # Bulk of the indicators, on the second HWDGE ring so
                    # it does not delay the emb stream.
                    nc.scalar.dma_start(
                        out=ind_t[:, IND1 * W : T * W],
                        in_=ind_d[:, IND1 * W : T * W],
                    )
                for t in range(t0, t1):
                    c0 = c0s[t]
                    nc.tensor.matmul(
                        ps[:, c0 : c0 + W],
                        lhsT=et[:, t * D : (t + 1) * D],
                        rhs=ind_t[:, t * W : (t + 1) * W],
                        start=False,
                        stop=False,
                    )
            nc.tensor.matmul(
                ps[:], lhsT=zero8[:], rhs=zero8[:], start=False, stop=True
            )
            nc.scalar.copy(out=out_t[:], in_=ps[:])
            nc.sync.dma_start(out=out_d[:], in_=out_t[:])

    nc.compile()
    return nc


def _prep_inputs(embeddings, labels):
    embeddings = np.ascontiguousarray(np.asarray(embeddings, dtype=np.float32))
    labels64 = np.asarray(labels).astype(np.int64)

    counts = np.bincount(labels64, minlength=C)
    buckets, R2 = _pack_classes(counts)
    T = -(-(NB_PER_CORE * R2) // 128)  # ceil
    ROWS = T * 128

    row_order = np.argsort(labels64, kind="stable")
    starts = np.concatenate([[0], np.cumsum(counts)])

    # fp32 -> fp16 -> fp8 via LUT (fast; ml_dtypes astype on 128M elems is slow)
    emb8u = _lut()[embeddings.astype(np.float16).view(np.uint16)]

    c0s = np.asarray(_schedule(T, R2))
    t_of_r = np.arange(ROWS) // 128
    one8 = np.float32(1.0).astype(F8).view(np.uint8)

    in_maps = []
    slot_to_class = np.full((CORES, 128), -1, dtype=np.int64)
    for k in range(CORES):
        idx = np.full(ROWS, -1, dtype=np.int64)
        slot = np.full(ROWS, -1, dtype=np.int64)
        for b_local, bucket in enumerate(
            buckets[k * NB_PER_CORE : (k + 1) * NB_PER_CORE]
        ):
            base = b_local * R2
            off = 0
            for side, c in enumerate(bucket):
                n = int(counts[c])
                idx[base + off : base + off + n] = row_order[
                    starts[c] : starts[c] + n
                ]
                slot[base + off : base + off + n] = 2 * b_local + side
                slot_to_class[k, 2 * b_local + side] = c
                off += n

        valid = idx >= 0
        e8 = np.zeros((ROWS, D), dtype=np.uint8)
        e8[valid] = emb8u[idx[valid]]
        emb_t = np.ascontiguousarray(
            e8.reshape(T, 128, D).transpose(1, 0, 2)
        ).reshape(128, T * D)

        j = slot - c0s[t_of_r]
        jv = j[valid]
        assert jv.min() >= 0 and jv.max() < W, "indicator window violated"
        ind = np.zeros((ROWS, W), dtype=np.uint8)
        ind[np.nonzero(valid)[0], jv] = one8
        ind_t = np.ascontiguousarray(
            ind.reshape(T, 128, W).transpose(1, 0, 2)
        ).reshape(128, T * W)

        in_maps.append(
            {"emb": emb_t.view(F8), "ind": ind_t.view(F8)}
        )
    return in_maps, slot_to_class, counts, T, R2


def _postprocess(results, slot_to_class, counts):
    sums = np.zeros((C, D), dtype=np.float64)
    for k, r in enumerate(results):
        out_k = r["out"].astype(np.float64)  # [128 dims, 128 slots]
        for s in range(128):
            c = slot_to_class[k, s]
            if c >= 0:
                sums[c] = out_k[:, s]
    means = sums / counts[:, None].astype(np.float64)
    mu = means.mean(axis=0)
    var = ((means - mu) ** 2).sum(axis=0) / (C - 1)
    return np.float32(-var.mean())


def kernel(embeddings, labels):
    global LAST_RESULT
    from concourse.bass_utils import run_bass_kernel_spmd

    in_maps, slot_to_class, counts, T, R2 = _prep_inputs(embeddings, labels)

    key = (T, R2)
    if key not in _cached_nc:
        _cached_nc.clear()
        _cached_nc[key] = _build_module(T, R2)
    nc = _cached_nc[key]

    res = run_bass_kernel_spmd(
        nc,
        in_maps,
        core_ids=list(range(CORES)),
        trace=TRACE,
        **TRACE_KWARGS,
    )
    LAST_RESULT = res
    return _postprocess(res.results, slot_to_class, counts)
